# revision 13
# baseline (speedup 1.0000x reference)
"""Trainium2 Bass kernel for nn_DKTAccum_no_tempo_Model (DKT with count-feature LSTM).

Strategy (8 NeuronCores, pure data parallel over batch, 16 rows/core):
  Phase A: stream x (fp16, channel-major, de-interleaved), compute
           embed = x @ Wx via PE, running interaction counts via DVE
           tensor_tensor_scan, extract correct/incorrect counts via
           pair-indicator multiply + ones-matmul, log1p on ACT.
  Phase B: LSTM, time-split into 8 segments per core with 64-step warmup
           (forget-gate decay keeps truncation error ~1e-6), all segments
           advanced in lockstep -> 127 serial rounds instead of 500.
  Phase C: output layer sigmoid(h @ Wo + bo) dotted with one-hot q.
"""
import sys

sys.path.insert(0, "/opt/trn_rl_repo")

import numpy as np

import concourse.bass as bass
import concourse.tile as tile
from concourse import bacc, mybir
from concourse.bass_utils import run_bass_kernel_spmd

# ---- problem constants -----------------------------------------------------
B, T, S = 128, 500, 200          # batch, seq, skills
E, H = 100, 100                  # embed dim, lstm hidden
NCORES = 8
BC = B // NCORES                 # 16 batch rows per core
G = 8                            # time segments per core
SEG = 63                         # real steps per segment (8*63 = 504 >= 500)
W = 64                           # warmup steps per segment
ROUNDS = W + SEG                 # 127 lockstep rounds
TP = G * SEG                     # padded T = 504
BLK = 128                        # columns per (batch, segment) block
NCOL = BC * G * BLK              # 16384 columns in Z_RHS / EMB
F16 = mybir.dt.float16
F32 = mybir.dt.float32
AOP = mybir.AluOpType

_cache = {}


def _build():
    nc = bacc.Bacc(
        "TRN2",
        target_bir_lowering=False,
        debug=False,
        enable_asserts=False,
        num_devices=NCORES,
    )
    xd_d = nc.dram_tensor("xd", [BC, 4, E, TP], F16, kind="ExternalInput")
    qh_d = nc.dram_tensor("qh", [BC, TP, S], F16, kind="ExternalInput")
    wxe_d = nc.dram_tensor("wxe", [4, E, 128], F16, kind="ExternalInput")
    rka_d = nc.dram_tensor("rka", [4, 104, 128], F16, kind="ExternalInput")
    ke_d = nc.dram_tensor("ke", [4, E, 128], F16, kind="ExternalInput")
    wob_d = nc.dram_tensor("wob", [104, S], F16, kind="ExternalInput")
    ones_d = nc.dram_tensor("onesrow", [1, NCOL], F16, kind="ExternalInput")
    ident_d = nc.dram_tensor("ident", [128, 128], F32, kind="ExternalInput")
    yout_d = nc.dram_tensor("yout", [BC * 4, 126], F32, kind="ExternalOutput")

    with tile.TileContext(nc) as tc:
        _emit(tc, nc, xd_d, qh_d, wxe_d, rka_d, ke_d, wob_d, ones_d, ident_d,
              yout_d)
    nc.compile()
    return nc


def _emit(tc, nc, xd_d, qh_d, wxe_d, rka_d, ke_d, wob_d, ones_d, ident_d,
          yout_d):
    from contextlib import ExitStack

    with ExitStack() as ctx:
        big = ctx.enter_context(tc.tile_pool(name="big", bufs=1))
        wpool = ctx.enter_context(tc.tile_pool(name="w", bufs=1))
        xdp = ctx.enter_context(tc.tile_pool(name="xd", bufs=8))
        cntp = ctx.enter_context(tc.tile_pool(name="cnt", bufs=8))
        s2p = ctx.enter_context(tc.tile_pool(name="s2", bufs=4))
        ep = ctx.enter_context(tc.tile_pool(name="emul", bufs=4))
        gp = ctx.enter_context(tc.tile_pool(name="gates", bufs=2))
        cp = ctx.enter_context(tc.tile_pool(name="cstate", bufs=2))
        qp = ctx.enter_context(tc.tile_pool(name="q", bufs=3))
        clp = ctx.enter_context(tc.tile_pool(name="clog", bufs=3))

        # ---- persistent tensors -------------------------------------------
        ZR = big.tile([128, NCOL], F16)       # h(0:100), cc/ic/ss(100:103), ones(103)
        EMB = big.tile([128, NCOL], F16)      # embed rows 0:100
        STG = big.tile([1, BC * 1920], F16)   # feats staging row, 64-col zero guard per block
        ZERO = big.tile([128, TP], F16)
        OUTS = big.tile([64, 128], F32)

        WXE = [wpool.tile([E, 128], F16, tag=f"wxe{k}", name=f"WXE{k}") for k in range(4)]
        RKA = [wpool.tile([104, 128], F16, tag=f"rka{k}", name=f"RKA{k}") for k in range(4)]
        KE = [wpool.tile([E, 128], F16, tag=f"ke{k}", name=f"KE{k}") for k in range(4)]
        WOB = wpool.tile([104, S], F16, tag="wob")
        ONES1 = wpool.tile([E, 1], F16, tag="ones1")
        IDN = wpool.tile([128, 128], F32, tag="idn")

        for k in range(4):
            nc.sync.dma_start(WXE[k][:], wxe_d.ap()[k])
            nc.sync.dma_start(RKA[k][:], rka_d.ap()[k])
            nc.sync.dma_start(KE[k][:], ke_d.ap()[k])
        nc.sync.dma_start(WOB[:], wob_d.ap()[:])
        nc.sync.dma_start(IDN[:], ident_d.ap()[:])

        nc.vector.memset(ZERO[:], 0.0)
        nc.vector.memset(ZR[:], 0.0)
        nc.vector.memset(EMB[:], 0.0)
        nc.vector.memset(STG[:], 0.0)
        nc.gpsimd.memset(ONES1[:], 1.0)
        nc.sync.dma_start(ZR[103:104, :], ones_d.ap()[:])

        embv = EMB[0:E, :].rearrange("p (b j q) -> p b j q", b=BC, j=G, q=BLK)

        # ---- phase A -------------------------------------------------------
        ctxA = ExitStack()
        pse = ctxA.enter_context(tc.tile_pool(name="pse", bufs=2, space="PSUM"))
        psx = ctxA.enter_context(tc.tile_pool(name="psx", bufs=1, space="PSUM"))
        pcc = pic = None
        for b in range(BC):
            jj, m = b % 4, b // 4
            xt = [xdp.tile([E, TP], F16, tag="xd", name=f"xt{b}_{k}") for k in range(4)]
            for k in range(4):
                nc.sync.dma_start(xt[k][:], xd_d.ap()[b, k])

            # embed: psum_e = sum_k WXE[k].T @ x[k]   -> [128(E pad), TP]
            pe = pse.tile([128, TP], F32, tag="pe")
            for k in range(4):
                nc.tensor.matmul(pe[:], WXE[k][:], xt[k][:], start=(k == 0),
                                 stop=(k == 3), skip_group_check=True)
            # scatter embed into EMB segment windows (fp16)
            nc.scalar.copy(embv[:, b, 0, 64:127], pe[0:E, 0:63])
            nc.scalar.copy(embv[:, b, 1, 1:127], pe[0:E, 0:126])
            for j in range(2, G):
                t0 = 63 * j - 64
                nc.scalar.copy(embv[:, b, j, 0:127], pe[0:E, t0:t0 + 127])

            # inclusive cumsum over t (the count RNN)
            ct = [cntp.tile([E, TP], F16, tag="cnt", name=f"ct{b}_{k}") for k in range(4)]
            for k in range(4):
                nc.vector.tensor_tensor_scan(
                    ct[k][:], xt[k][:], ZERO[0:E, :], 0.0,
                    op0=AOP.add, op1=AOP.add)

            # pair indicator s2 = x_corr + x_incorr
            s2 = [s2p.tile([E, TP], F16, tag="s2", name=f"s2_{b}_{k}") for k in range(2)]
            for k in range(2):
                nc.vector.tensor_tensor(s2[k][:], xt[k][:], xt[k + 2][:],
                                        op=AOP.add)

            # raw count rows -> psum partition 0 (per batch row)
            pcc = psx.tile([1, TP], F32, tag="pcc", name=f"pcc{b}")
            pic = psx.tile([1, TP], F32, tag="pic", name=f"pic{b}")
            for k in range(2):
                em = ep.tile([E, TP], F16, tag="emul", name=f"em{b}_{k}")
                nc.vector.tensor_tensor(em[:], ct[k][:], s2[k][:], op=AOP.mult)
                nc.tensor.matmul(pcc[:], ONES1[:], em[:],
                                 start=(k == 0), stop=(k == 1),
                                 skip_group_check=True)
            for k in range(2):
                em = ep.tile([E, TP], F16, tag="emul", name=f"em{b}_{k}")
                nc.vector.tensor_tensor(em[:], ct[k + 2][:], s2[k][:],
                                        op=AOP.mult)
                nc.tensor.matmul(pic[:], ONES1[:], em[:],
                                 start=(k == 0), stop=(k == 1),
                                 skip_group_check=True)

            # log1p -> staging row (t-layout, 64-col zero guard per block)
            sv = STG[0:1, 1920 * b:1920 * (b + 1)]
            nc.scalar.activation(sv[:, 64:64 + TP], pcc[:],
                                 mybir.ActivationFunctionType.Ln,
                                 bias=1.0, scale=1.0)
            nc.scalar.activation(sv[:, 704:704 + TP], pic[:],
                                 mybir.ActivationFunctionType.Ln,
                                 bias=1.0, scale=1.0)
            nc.vector.tensor_tensor(sv[:, 1344:1344 + TP], sv[:, 64:64 + TP],
                                    sv[:, 704:704 + TP], op=AOP.add)

        # feats staging -> ZR rows 100/101/102: one 3-dim DMA per (b, feat);
        # overlapping strided reads do the segment-window expansion, the
        # guard columns supply zeros for tau < 0.
        zrv = ZR[:].rearrange("p (b j q) -> p b j q", b=BC, j=G, q=BLK)
        for b in range(BC):
            for f in range(3):
                c0 = 1920 * b + 640 * f
                s3 = STG[0:1, c0:c0 + 640]
                srcv = bass.AP(
                    s3.tensor, s3.offset,
                    [s3.ap[0], [63, G], [1, 127]])
                nc.sync.dma_start(zrv[100 + f:101 + f, b, :, 0:127], srcv)

        ctxA.close()

        # ---- phase B: lockstep segmented LSTM -----------------------------
        ctxB = ExitStack()
        psz = ctxB.enter_context(tc.tile_pool(name="psz", bufs=3, space="PSUM"))
        zrr = ZR[:].rearrange("p (b j q) -> p b j q", b=BC, j=G, q=BLK)
        zru = ZR[:].rearrange("p (u q) -> p u q", u=BC * G, q=BLK)
        embu = EMB[:].rearrange("p (u q) -> p u q", u=BC * G, q=BLK)
        c_prev = cp.tile([H, 128], F32, tag="c")
        nc.vector.memset(c_prev[:], 0.0)
        for r in range(ROUNDS):
            pz = psz.tile([128, 512], F32, tag="pz")
            for g in range(4):
                nc.tensor.matmul(pz[:, 128 * g:128 * (g + 1)],
                                 KE[g][:], embu[0:E, :, r],
                                 start=(g == 0), stop=False,
                                 skip_group_check=True)
            for g in range(4):
                nc.tensor.matmul(pz[:, 128 * g:128 * (g + 1)],
                                 RKA[g][:], zru[0:104, :, r],
                                 start=False, stop=(g == 3),
                                 skip_group_check=True)
            sig = gp.tile([H, 384], F32, tag="sig")
            tg = gp.tile([H, 128], F32, tag="tg")
            nc.scalar.activation(sig[:], pz[0:H, 0:384],
                                 mybir.ActivationFunctionType.Sigmoid)
            nc.scalar.activation(tg[:], pz[0:H, 384:512],
                                 mybir.ActivationFunctionType.Tanh)
            u = gp.tile([H, 128], F32, tag="u")
            v = gp.tile([H, 128], F32, tag="v")
            nc.vector.tensor_tensor(u[:], sig[:, 128:256], c_prev[:],
                                    op=AOP.mult)
            nc.vector.tensor_tensor(v[:], sig[:, 0:128], tg[:], op=AOP.mult)
            c_new = cp.tile([H, 128], F32, tag="c")
            nc.vector.tensor_tensor(c_new[:], u[:], v[:], op=AOP.add)
            tc_t = gp.tile([H, 128], F32, tag="tc")
            nc.scalar.activation(tc_t[:], c_new[:],
                                 mybir.ActivationFunctionType.Tanh)
            nc.vector.tensor_tensor(zru[0:H, :, r + 1], sig[:, 256:384],
                                    tc_t[:], op=AOP.mult)
            c_prev = c_new

        ctxB.close()

        # ---- phase C: output layer ----------------------------------------
        # even segments and odd segments in separate 63-row pipelines
        ctxC = ExitStack()
        psc = ctxC.enter_context(tc.tile_pool(name="psc", bufs=2, space="PSUM"))
        qv = qh_d.ap().rearrange("b (jp s w) c -> b jp s w c", jp=4, s=2, w=63)
        YE = big.tile([63, BC * 4], F32, name="YE")
        YO = big.tile([63, BC * 4], F32, name="YO")
        nc.vector.memset(YE[:], 0.0)
        nc.vector.memset(YO[:], 0.0)
        for b in range(BC):
            qte = qp.tile([63, 4 * S], F16, tag="qte", name=f"qte{b}")
            qto = qp.tile([63, 4 * S], F16, tag="qto", name=f"qto{b}")
            nc.sync.dma_start(
                qte[:].rearrange("p (jp c) -> p jp c", jp=4),
                qv[b, :, 0, :, :].transpose([1, 0, 2]))
            nc.sync.dma_start(
                qto[:].rearrange("p (jp c) -> p jp c", jp=4),
                qv[b, :, 1, :, :].transpose([1, 0, 2]))
            for par, qt, yall in ((0, qte, YE), (1, qto, YO)):
                # two psum tiles: a matmul output must not straddle a bank
                for hh in range(2):
                    pc = psc.tile([63, 2 * S], F32, tag="pc",
                                  name=f"pc{b}_{par}_{hh}")
                    for k in range(2):
                        jp = 2 * hh + k
                        nc.tensor.matmul(pc[:, S * k:S * (k + 1)],
                                         zrr[0:104, b, 2 * jp + par, 65:128],
                                         WOB[:], start=True, stop=True,
                                         skip_group_check=True)
                    sl = clp.tile([63, 2 * S], F16, tag="sl",
                                  name=f"sl{b}_{par}_{hh}")
                    nc.scalar.activation(sl[:], pc[:],
                                         mybir.ActivationFunctionType.Sigmoid)
                    for k in range(2):
                        jp = 2 * hh + k
                        scr = clp.tile([63, S], F16, tag="scr",
                                       name=f"scr{b}_{par}_{jp}")
                        nc.vector.scalar_tensor_tensor(
                            scr[:], sl[:, S * k:S * (k + 1)], 1.0,
                            qt[:, S * jp:S * (jp + 1)],
                            op0=AOP.mult, op1=AOP.mult,
                            accum_out=yall[0:63, 4 * b + jp:4 * b + jp + 1])

        pte = psc.tile([64, 64], F32, tag="pt", name="pte")
        pto = psc.tile([64, 64], F32, tag="pt", name="pto")
        nc.tensor.transpose(pte[:, 0:63], YE[:, 0:64], IDN[0:63, 0:63])
        nc.tensor.transpose(pto[:, 0:63], YO[:, 0:64], IDN[0:63, 0:63])
        nc.scalar.copy(OUTS[:, 0:63], pte[:, 0:63])
        nc.scalar.copy(OUTS[:, 63:126], pto[:, 0:63])
        nc.sync.dma_start(yout_d.ap()[:], OUTS[:, 0:126])
        ctxC.close()


# ---- host side -------------------------------------------------------------
def _prep(inputs):
    x = np.asarray(inputs["x"], np.float32)
    q = np.asarray(inputs["q"], np.float32)
    Wx = np.asarray(inputs["Wx"], np.float32)
    bx = np.asarray(inputs["bx"], np.float32)
    lstm_k = np.asarray(inputs["lstm_k"], np.float32)
    lstm_rk = np.asarray(inputs["lstm_rk"], np.float32)
    lstm_b = np.asarray(inputs["lstm_b"], np.float32)
    Wo = np.asarray(inputs["Wo"], np.float32)
    bo = np.asarray(inputs["bo"], np.float32)

    # channel de-interleave: deint[..., skill + 200*bit] = orig[..., 2*skill+bit]
    perm = np.empty(2 * S, np.int64)
    sk = np.arange(S)
    perm[sk] = 2 * sk
    perm[S + sk] = 2 * sk + 1

    xd = x[:, :, perm].transpose(0, 2, 1)                 # [B, 400, T]
    xdp = np.zeros((B, 4, E, TP), np.float16)
    xdp[:, :, :, :T] = xd.reshape(B, 4, E, T).astype(np.float16)

    qhp = np.zeros((B, TP, S), np.float16)
    qhp[:, :T, :] = q.astype(np.float16)

    # gate reorder [i,f,g,o] -> [i,f,o,g]
    gperm = np.concatenate([np.arange(H), H + np.arange(H),
                            3 * H + np.arange(H), 2 * H + np.arange(H)])
    k_r = lstm_k[:, gperm]
    rk_r = lstm_rk[:, gperm]
    b_r = lstm_b[gperm]
    Wxd = Wx[perm]

    bias_row = bx @ k_r[:E] + b_r

    wxe = np.zeros((4, E, 128), np.float16)
    wxe[:, :, :E] = Wxd.reshape(4, E, E).astype(np.float16)

    rka = np.zeros((4, 104, 128), np.float16)
    for g in range(4):
        cols = slice(100 * g, 100 * (g + 1))
        rka[g, 0:H, 0:100] = rk_r[:, cols].astype(np.float16)
        rka[g, 100, 0:100] = k_r[E, cols].astype(np.float16)
        rka[g, 101, 0:100] = k_r[E + 1, cols].astype(np.float16)
        rka[g, 102, 0:100] = k_r[E + 2, cols].astype(np.float16)
        rka[g, 103, 0:100] = bias_row[cols].astype(np.float16)

    ke = np.zeros((4, E, 128), np.float16)
    for g in range(4):
        ke[g, :, 0:100] = k_r[:E, 100 * g:100 * (g + 1)].astype(np.float16)

    wob = np.zeros((104, S), np.float16)
    wob[0:H] = Wo.astype(np.float16)
    wob[103] = bo.astype(np.float16)

    onesrow = np.zeros((1, NCOL), np.float16)
    qq = np.arange(BLK)
    for j in range(G):
        tau = 63 * j - 64 + qq
        valid = (tau >= 0) & (tau < T)
        for b in range(BC):
            base = b * G * BLK + j * BLK
            onesrow[0, base:base + BLK][valid] = 1.0

    ident = np.eye(128, dtype=np.float32)
    return xdp, qhp, wxe, rka, ke, wob, onesrow, ident


def kernel(**inputs):
    if "nc" not in _cache:
        _cache["nc"] = _build()
    nc = _cache["nc"]

    xdp, qhp, wxe, rka, ke, wob, onesrow, ident = _prep(inputs)

    in_maps = []
    for c in range(NCORES):
        sl = slice(c * BC, (c + 1) * BC)
        in_maps.append({
            "xd": np.ascontiguousarray(xdp[sl]),
            "qh": np.ascontiguousarray(qhp[sl]),
            "wxe": wxe, "rka": rka, "ke": ke, "wob": wob,
            "onesrow": onesrow, "ident": ident,
        })

    res = run_bass_kernel_spmd(nc, in_maps, core_ids=list(range(NCORES)))

    y = np.zeros((B, T, 1), np.float32)
    for c in range(NCORES):
        yo = np.asarray(res.results[c]["yout"])     # [64, 126]
        yo = yo.reshape(BC, 4 * 126)[:, :T]
        y[c * BC:(c + 1) * BC, :, 0] = yo
    return y


# revision 14
# speedup vs baseline: 5.2740x; 5.2740x over previous
"""Trainium2 Bass kernel for nn_DKTAccum_no_tempo_Model (DKT with count-feature LSTM).

Strategy (8 NeuronCores, pure data parallel over batch, 16 rows/core):
  Phase A: stream x (fp16, channel-major, de-interleaved), compute
           embed = x @ Wx via PE, running interaction counts via DVE
           tensor_tensor_scan, extract correct/incorrect counts via
           pair-indicator multiply + ones-matmul, log1p on ACT.
  Phase B: LSTM, time-split into 8 segments per core with 64-step warmup
           (forget-gate decay keeps truncation error ~1e-6), all segments
           advanced in lockstep -> 127 serial rounds instead of 500.
  Phase C: output layer sigmoid(h @ Wo + bo) dotted with one-hot q.
"""
import sys

sys.path.insert(0, "/opt/trn_rl_repo")

import numpy as np

import concourse.bass as bass
import concourse.tile as tile
from concourse import bacc, mybir
from concourse.bass_utils import run_bass_kernel_spmd

# ---- problem constants -----------------------------------------------------
B, T, S = 128, 500, 200          # batch, seq, skills
E, H = 100, 100                  # embed dim, lstm hidden
NCORES = 8
BC = B // NCORES                 # 16 batch rows per core
G = 8                            # time segments per core
SEG = 63                         # real steps per segment (8*63 = 504 >= 500)
W = 64                           # warmup steps per segment
ROUNDS = W + SEG                 # 127 lockstep rounds
TP = G * SEG                     # padded T = 504
BLK = 128                        # columns per (batch, segment) block
NCOL = BC * G * BLK              # 16384 columns in Z_RHS / EMB
F16 = mybir.dt.float16
F32 = mybir.dt.float32
AOP = mybir.AluOpType

_cache = {}


def _build():
    nc = bacc.Bacc(
        "TRN2",
        target_bir_lowering=False,
        debug=False,
        enable_asserts=False,
        num_devices=NCORES,
    )
    xd_d = nc.dram_tensor("xd", [BC, E, 4 * TP], F16, kind="ExternalInput")
    qh_d = nc.dram_tensor("qh", [BC, 2, 63, 4 * S], F16, kind="ExternalInput")
    wxe_d = nc.dram_tensor("wxe", [4, E, 128], F16, kind="ExternalInput")
    rka_d = nc.dram_tensor("rka", [4, 104, 128], F16, kind="ExternalInput")
    ke_d = nc.dram_tensor("ke", [4, E, 128], F16, kind="ExternalInput")
    wob_d = nc.dram_tensor("wob", [104, S], F16, kind="ExternalInput")
    ones_d = nc.dram_tensor("onesrow", [1, NCOL], F16, kind="ExternalInput")
    ident_d = nc.dram_tensor("ident", [128, 128], F32, kind="ExternalInput")
    yout_d = nc.dram_tensor("yout", [BC * 4, 126], F32, kind="ExternalOutput")

    with tile.TileContext(nc) as tc:
        _emit(tc, nc, xd_d, qh_d, wxe_d, rka_d, ke_d, wob_d, ones_d, ident_d,
              yout_d)
    nc.compile()
    return nc


def _emit(tc, nc, xd_d, qh_d, wxe_d, rka_d, ke_d, wob_d, ones_d, ident_d,
          yout_d):
    from contextlib import ExitStack

    with ExitStack() as ctx:
        big = ctx.enter_context(tc.tile_pool(name="big", bufs=1))
        wpool = ctx.enter_context(tc.tile_pool(name="w", bufs=1))
        xdp = ctx.enter_context(tc.tile_pool(name="xd", bufs=2))
        cntp = ctx.enter_context(tc.tile_pool(name="cnt", bufs=8))
        s2p = ctx.enter_context(tc.tile_pool(name="s2", bufs=4))
        ep = ctx.enter_context(tc.tile_pool(name="emul", bufs=4))
        gp = ctx.enter_context(tc.tile_pool(name="gates", bufs=2))
        cp = ctx.enter_context(tc.tile_pool(name="cstate", bufs=2))
        qp = ctx.enter_context(tc.tile_pool(name="q", bufs=3))
        clp = ctx.enter_context(tc.tile_pool(name="clog", bufs=3))

        # ---- persistent tensors -------------------------------------------
        ZR = big.tile([128, NCOL], F16)       # h(0:100), cc/ic/ss(100:103), ones(103)
        EMB = big.tile([128, NCOL], F16)      # embed rows 0:100
        STG = big.tile([1, BC * 1920], F16)   # feats staging row, 64-col zero guard per block
        ZERO = big.tile([128, TP], F16)
        OUTS = big.tile([64, 128], F32)

        WXE = [wpool.tile([E, 128], F16, tag=f"wxe{k}", name=f"WXE{k}") for k in range(4)]
        RKA = [wpool.tile([104, 128], F16, tag=f"rka{k}", name=f"RKA{k}") for k in range(4)]
        KE = [wpool.tile([E, 128], F16, tag=f"ke{k}", name=f"KE{k}") for k in range(4)]
        WOB = wpool.tile([104, S], F16, tag="wob")
        ONES1 = wpool.tile([E, 1], F16, tag="ones1")
        IDN = wpool.tile([128, 128], F32, tag="idn")

        for k in range(4):
            nc.sync.dma_start(WXE[k][:], wxe_d.ap()[k])
            nc.sync.dma_start(RKA[k][:], rka_d.ap()[k])
            nc.sync.dma_start(KE[k][:], ke_d.ap()[k])
        nc.sync.dma_start(WOB[:], wob_d.ap()[:])
        nc.sync.dma_start(IDN[:], ident_d.ap()[:])

        nc.vector.memset(ZERO[:], 0.0)
        nc.vector.memset(ZR[:], 0.0)
        nc.vector.memset(EMB[:], 0.0)
        nc.vector.memset(STG[:], 0.0)
        nc.gpsimd.memset(ONES1[:], 1.0)
        nc.sync.dma_start(ZR[103:104, :], ones_d.ap()[:])

        embv = EMB[0:E, :].rearrange("p (b j q) -> p b j q", b=BC, j=G, q=BLK)

        # ---- phase A -------------------------------------------------------
        ctxA = ExitStack()
        pse = ctxA.enter_context(tc.tile_pool(name="pse", bufs=2, space="PSUM"))
        psx = ctxA.enter_context(tc.tile_pool(name="psx", bufs=1, space="PSUM"))
        pcc = pic = None
        for b in range(BC):
            jj, m = b % 4, b // 4
            XT = xdp.tile([E, 4 * TP], F16, tag="xd", name=f"xt{b}")
            nc.sync.dma_start(XT[:], xd_d.ap()[b])
            xt = [XT[:, TP * k:TP * (k + 1)] for k in range(4)]

            # embed: psum_e = sum_k WXE[k].T @ x[k]   -> [128(E pad), TP]
            pe = pse.tile([128, TP], F32, tag="pe")
            for k in range(4):
                nc.tensor.matmul(pe[:], WXE[k][:], xt[k], start=(k == 0),
                                 stop=(k == 3), skip_group_check=True)
            # scatter embed into EMB segment windows (fp16)
            nc.scalar.copy(embv[:, b, 0, 64:127], pe[0:E, 0:63])
            nc.scalar.copy(embv[:, b, 1, 1:127], pe[0:E, 0:126])
            for j in range(2, G):
                t0 = 63 * j - 64
                nc.scalar.copy(embv[:, b, j, 0:127], pe[0:E, t0:t0 + 127])

            # inclusive cumsum over t (the count RNN)
            ct = [cntp.tile([E, TP], F16, tag="cnt", name=f"ct{b}_{k}") for k in range(4)]
            for k in range(4):
                nc.vector.tensor_tensor_scan(
                    ct[k][:], xt[k], ZERO[0:E, :], 0.0,
                    op0=AOP.add, op1=AOP.add)

            # pair indicator s2 = x_corr + x_incorr
            s2 = [s2p.tile([E, TP], F16, tag="s2", name=f"s2_{b}_{k}") for k in range(2)]
            for k in range(2):
                nc.vector.tensor_tensor(s2[k][:], xt[k], xt[k + 2],
                                        op=AOP.add)

            # raw count rows -> psum partition 0 (per batch row)
            pcc = psx.tile([1, TP], F32, tag="pcc", name=f"pcc{b}")
            pic = psx.tile([1, TP], F32, tag="pic", name=f"pic{b}")
            for k in range(2):
                em = ep.tile([E, TP], F16, tag="emul", name=f"em{b}_{k}")
                nc.vector.tensor_tensor(em[:], ct[k][:], s2[k][:], op=AOP.mult)
                nc.tensor.matmul(pcc[:], ONES1[:], em[:],
                                 start=(k == 0), stop=(k == 1),
                                 skip_group_check=True)
            for k in range(2):
                em = ep.tile([E, TP], F16, tag="emul", name=f"em{b}_{k}")
                nc.vector.tensor_tensor(em[:], ct[k + 2][:], s2[k][:],
                                        op=AOP.mult)
                nc.tensor.matmul(pic[:], ONES1[:], em[:],
                                 start=(k == 0), stop=(k == 1),
                                 skip_group_check=True)

            # log1p -> staging row (t-layout, 64-col zero guard per block)
            sv = STG[0:1, 1920 * b:1920 * (b + 1)]
            nc.scalar.activation(sv[:, 64:64 + TP], pcc[:],
                                 mybir.ActivationFunctionType.Ln,
                                 bias=1.0, scale=1.0)
            nc.scalar.activation(sv[:, 704:704 + TP], pic[:],
                                 mybir.ActivationFunctionType.Ln,
                                 bias=1.0, scale=1.0)
            nc.vector.tensor_tensor(sv[:, 1344:1344 + TP], sv[:, 64:64 + TP],
                                    sv[:, 704:704 + TP], op=AOP.add)

        # feats staging -> ZR rows 100/101/102: one 3-dim DMA per (b, feat);
        # overlapping strided reads do the segment-window expansion, the
        # guard columns supply zeros for tau < 0.
        zrv = ZR[:].rearrange("p (b j q) -> p b j q", b=BC, j=G, q=BLK)
        for b in range(BC):
            for f in range(3):
                c0 = 1920 * b + 640 * f
                s3 = STG[0:1, c0:c0 + 640]
                srcv = bass.AP(
                    s3.tensor, s3.offset,
                    [s3.ap[0], [63, G], [1, 127]])
                nc.sync.dma_start(zrv[100 + f:101 + f, b, :, 0:127], srcv)

        ctxA.close()

        # ---- phase B: lockstep segmented LSTM -----------------------------
        ctxB = ExitStack()
        psz = ctxB.enter_context(tc.tile_pool(name="psz", bufs=3, space="PSUM"))
        zrr = ZR[:].rearrange("p (b j q) -> p b j q", b=BC, j=G, q=BLK)
        zru = ZR[:].rearrange("p (u q) -> p u q", u=BC * G, q=BLK)
        embu = EMB[:].rearrange("p (u q) -> p u q", u=BC * G, q=BLK)
        c_prev = cp.tile([H, 128], F32, tag="c")
        nc.vector.memset(c_prev[:], 0.0)
        for r in range(ROUNDS):
            pz = psz.tile([128, 512], F32, tag="pz")
            for g in range(4):
                nc.tensor.matmul(pz[:, 128 * g:128 * (g + 1)],
                                 KE[g][:], embu[0:E, :, r],
                                 start=(g == 0), stop=False,
                                 skip_group_check=True)
            for g in range(4):
                nc.tensor.matmul(pz[:, 128 * g:128 * (g + 1)],
                                 RKA[g][:], zru[0:104, :, r],
                                 start=False, stop=(g == 3),
                                 skip_group_check=True)
            sig = gp.tile([H, 384], F32, tag="sig")
            tg = gp.tile([H, 128], F32, tag="tg")
            nc.scalar.activation(sig[:], pz[0:H, 0:384],
                                 mybir.ActivationFunctionType.Sigmoid)
            nc.scalar.activation(tg[:], pz[0:H, 384:512],
                                 mybir.ActivationFunctionType.Tanh)
            u = gp.tile([H, 128], F32, tag="u")
            v = gp.tile([H, 128], F32, tag="v")
            nc.vector.tensor_tensor(u[:], sig[:, 128:256], c_prev[:],
                                    op=AOP.mult)
            nc.vector.tensor_tensor(v[:], sig[:, 0:128], tg[:], op=AOP.mult)
            c_new = cp.tile([H, 128], F32, tag="c")
            nc.vector.tensor_tensor(c_new[:], u[:], v[:], op=AOP.add)
            tc_t = gp.tile([H, 128], F32, tag="tc")
            nc.scalar.activation(tc_t[:], c_new[:],
                                 mybir.ActivationFunctionType.Tanh)
            nc.vector.tensor_tensor(zru[0:H, :, r + 1], sig[:, 256:384],
                                    tc_t[:], op=AOP.mult)
            c_prev = c_new

        ctxB.close()

        # ---- phase C: output layer ----------------------------------------
        # even segments and odd segments in separate 63-row pipelines
        ctxC = ExitStack()
        psc = ctxC.enter_context(tc.tile_pool(name="psc", bufs=2, space="PSUM"))

        YE = big.tile([63, BC * 4], F32, name="YE")
        YO = big.tile([63, BC * 4], F32, name="YO")
        nc.vector.memset(YE[:], 0.0)
        nc.vector.memset(YO[:], 0.0)
        for b in range(BC):
            qte = qp.tile([63, 4 * S], F16, tag="qte", name=f"qte{b}")
            qto = qp.tile([63, 4 * S], F16, tag="qto", name=f"qto{b}")
            nc.sync.dma_start(qte[:], qh_d.ap()[b, 0])
            nc.sync.dma_start(qto[:], qh_d.ap()[b, 1])
            for par, qt, yall in ((0, qte, YE), (1, qto, YO)):
                # two psum tiles: a matmul output must not straddle a bank
                for hh in range(2):
                    pc = psc.tile([63, 2 * S], F32, tag="pc",
                                  name=f"pc{b}_{par}_{hh}")
                    for k in range(2):
                        jp = 2 * hh + k
                        nc.tensor.matmul(pc[:, S * k:S * (k + 1)],
                                         zrr[0:104, b, 2 * jp + par, 65:128],
                                         WOB[:], start=True, stop=True,
                                         skip_group_check=True)
                    sl = clp.tile([63, 2 * S], F16, tag="sl",
                                  name=f"sl{b}_{par}_{hh}")
                    nc.scalar.activation(sl[:], pc[:],
                                         mybir.ActivationFunctionType.Sigmoid)
                    for k in range(2):
                        jp = 2 * hh + k
                        scr = clp.tile([63, S], F16, tag="scr",
                                       name=f"scr{b}_{par}_{jp}")
                        nc.vector.scalar_tensor_tensor(
                            scr[:], sl[:, S * k:S * (k + 1)], 1.0,
                            qt[:, S * jp:S * (jp + 1)],
                            op0=AOP.mult, op1=AOP.mult,
                            accum_out=yall[0:63, 4 * b + jp:4 * b + jp + 1])

        pte = psc.tile([64, 64], F32, tag="pt", name="pte")
        pto = psc.tile([64, 64], F32, tag="pt", name="pto")
        nc.tensor.transpose(pte[:, 0:63], YE[:, 0:64], IDN[0:63, 0:63])
        nc.tensor.transpose(pto[:, 0:63], YO[:, 0:64], IDN[0:63, 0:63])
        nc.scalar.copy(OUTS[:, 0:63], pte[:, 0:63])
        nc.scalar.copy(OUTS[:, 63:126], pto[:, 0:63])
        nc.sync.dma_start(yout_d.ap()[:], OUTS[:, 0:126])
        ctxC.close()


# ---- host side -------------------------------------------------------------
def _prep(inputs):
    x = np.asarray(inputs["x"], np.float32)
    q = np.asarray(inputs["q"], np.float32)
    Wx = np.asarray(inputs["Wx"], np.float32)
    bx = np.asarray(inputs["bx"], np.float32)
    lstm_k = np.asarray(inputs["lstm_k"], np.float32)
    lstm_rk = np.asarray(inputs["lstm_rk"], np.float32)
    lstm_b = np.asarray(inputs["lstm_b"], np.float32)
    Wo = np.asarray(inputs["Wo"], np.float32)
    bo = np.asarray(inputs["bo"], np.float32)

    # channel de-interleave: deint[..., skill + 200*bit] = orig[..., 2*skill+bit]
    perm = np.empty(2 * S, np.int64)
    sk = np.arange(S)
    perm[sk] = 2 * sk
    perm[S + sk] = 2 * sk + 1

    xd = x[:, :, perm].transpose(0, 2, 1)                 # [B, 400, T]
    xdp = np.zeros((B, E, 4, TP), np.float16)
    xdp[:, :, :, :T] = xd.reshape(B, 4, E, T).transpose(0, 2, 1, 3).astype(
        np.float16)
    xdp = xdp.reshape(B, E, 4 * TP)

    # q pre-arranged: [B, seg-half, 63, jp*S] so device loads are contiguous
    qtmp = np.zeros((B, TP, S), np.float16)
    qtmp[:, :T, :] = q.astype(np.float16)
    qhp = np.ascontiguousarray(
        qtmp.reshape(B, 4, 2, 63, S).transpose(0, 2, 3, 1, 4).reshape(
            B, 2, 63, 4 * S))

    # gate reorder [i,f,g,o] -> [i,f,o,g]
    gperm = np.concatenate([np.arange(H), H + np.arange(H),
                            3 * H + np.arange(H), 2 * H + np.arange(H)])
    k_r = lstm_k[:, gperm]
    rk_r = lstm_rk[:, gperm]
    b_r = lstm_b[gperm]
    Wxd = Wx[perm]

    bias_row = bx @ k_r[:E] + b_r

    wxe = np.zeros((4, E, 128), np.float16)
    wxe[:, :, :E] = Wxd.reshape(4, E, E).astype(np.float16)

    rka = np.zeros((4, 104, 128), np.float16)
    for g in range(4):
        cols = slice(100 * g, 100 * (g + 1))
        rka[g, 0:H, 0:100] = rk_r[:, cols].astype(np.float16)
        rka[g, 100, 0:100] = k_r[E, cols].astype(np.float16)
        rka[g, 101, 0:100] = k_r[E + 1, cols].astype(np.float16)
        rka[g, 102, 0:100] = k_r[E + 2, cols].astype(np.float16)
        rka[g, 103, 0:100] = bias_row[cols].astype(np.float16)

    ke = np.zeros((4, E, 128), np.float16)
    for g in range(4):
        ke[g, :, 0:100] = k_r[:E, 100 * g:100 * (g + 1)].astype(np.float16)

    wob = np.zeros((104, S), np.float16)
    wob[0:H] = Wo.astype(np.float16)
    wob[103] = bo.astype(np.float16)

    onesrow = np.zeros((1, NCOL), np.float16)
    qq = np.arange(BLK)
    for j in range(G):
        tau = 63 * j - 64 + qq
        valid = (tau >= 0) & (tau < T)
        for b in range(BC):
            base = b * G * BLK + j * BLK
            onesrow[0, base:base + BLK][valid] = 1.0

    ident = np.eye(128, dtype=np.float32)
    return xdp, qhp, wxe, rka, ke, wob, onesrow, ident


def kernel(**inputs):
    if "nc" not in _cache:
        _cache["nc"] = _build()
    nc = _cache["nc"]

    xdp, qhp, wxe, rka, ke, wob, onesrow, ident = _prep(inputs)

    in_maps = []
    for c in range(NCORES):
        sl = slice(c * BC, (c + 1) * BC)
        in_maps.append({
            "xd": np.ascontiguousarray(xdp[sl]),
            "qh": np.ascontiguousarray(qhp[sl]),
            "wxe": wxe, "rka": rka, "ke": ke, "wob": wob,
            "onesrow": onesrow, "ident": ident,
        })

    res = run_bass_kernel_spmd(nc, in_maps, core_ids=list(range(NCORES)))

    y = np.zeros((B, T, 1), np.float32)
    for c in range(NCORES):
        yo = np.asarray(res.results[c]["yout"])     # [64, 126]
        yo = yo.reshape(BC, 4 * 126)[:, :T]
        y[c * BC:(c + 1) * BC, :, 0] = yo
    return y
